# revision 36
# baseline (speedup 1.0000x reference)
"""Multi-head differential attention on 8 Trainium2 NeuronCores.

Sharding: core c -> batch c//4, head-group c%4 (4 of 16 heads).
Per core: QKV projection for its heads (pair-1 q/k and v sprinkled into
pair-0's exp-bound attention loop), k-major attention (scores via
row-group-packed 64-partition matmul pairs; softmax denominators from a
ones-row appended to V), per-(batch,head,qt-chunk) softmax normalization
pipelined inside the attention loop, GroupNorm statistics via bn_stats
with a DVE Newton rsqrt (no scalar-engine table switch).

Raw (pre-GroupNorm) z is AllGathered in eight per-qt [128,512] chunks
pipelined across both pairs' attention so the slow collective fabric
streams continuously.  Each pair's GN scalars (mean/rstd per head)
travel as bitcast payload: pair 0's ride its last chunk, pair 1's go in
a tiny dedicated gather fired before the last chunk so the Wo-scaling
fold overlaps the final transfer.  The receiver folds (bv-M)*r into a
scaled Wo and a bias row; the out-projection accumulates pair-0 chunks
while the last gather is in flight and finishes per seq-tile.

Host side folds: lambda and softmax scale into Wq/bq; GroupNorm affine
into Wo/bo.  x is pre-transposed per batch and cast to bf16.
"""

import numpy as np
import ml_dtypes

B, S, D, H, DH = 2, 2048, 1024, 16, 64
HPC = 4            # heads per core
CW = HPC * DH      # attention columns per core (256)
EPS = 1e-5
LAMBDA_INIT = 0.8
N_CORES = 8
SCC = 8            # scalar payload columns (4 f32 as 8 bf16)
RSQRT_MAGIC = 1.32118221e+19   # f32 with bits 0x5f3759df

NQT = 4            # query tiles of 512
QT = 512
NKT = 16           # key tiles of 128
NDC = 8            # d-chunks of 128
RG = [[0, 1, 2, 3], [4, 5, 6, 7]]

_cache = {}


def _build(with_collective=True, debug=False):
    from contextlib import ExitStack
    import concourse.bass as bass
    from concourse import bacc
    import concourse.tile as tile
    import concourse.mybir as mybir

    f32 = mybir.dt.float32
    i32 = mybir.dt.int32
    bf16 = mybir.dt.bfloat16
    AF = mybir.ActivationFunctionType
    ALU = mybir.AluOpType

    nc = bacc.Bacc("TRN2", target_bir_lowering=False, debug=False,
                   num_devices=N_CORES)

    xt_d = nc.dram_tensor("xt", [D, S], bf16, kind="ExternalInput")
    # weights host-prearranged to [partition, chunk*cols] for contiguous DMA
    wq_d = nc.dram_tensor("wq", [128, NDC * CW], bf16, kind="ExternalInput")
    wk_d = nc.dram_tensor("wk", [128, NDC * CW], bf16, kind="ExternalInput")
    wv_d = nc.dram_tensor("wv", [128, NDC * CW], bf16, kind="ExternalInput")
    wo_d = nc.dram_tensor("wo", [128, NDC * CW], bf16, kind="ExternalInput")
    bqp_d = nc.dram_tensor("bqp", [128, 2], f32, kind="ExternalInput")
    bkp_d = nc.dram_tensor("bkp", [128, 2], f32, kind="ExternalInput")
    bvh_d = nc.dram_tensor("bvh", [64, HPC], f32, kind="ExternalInput")
    bvo_d = nc.dram_tensor("bvo", [128, 2 * HPC], f32, kind="ExternalInput")
    bo_d = nc.dram_tensor("bo", [CW], bf16, kind="ExternalInput")
    y_d = nc.dram_tensor("y", [2, 128, S], bf16, kind="ExternalOutput")

    # per-(pair,qt) chunk gathers; qt3 chunks carry the payload columns
    def chunk_w(t, q):
        return QT + (SCC if q == 3 else 0)
    agc_in = [[nc.dram_tensor(f"agc_in{t}{q}", [128, chunk_w(t, q)], bf16)
               for q in range(NQT)] for t in range(2)]
    agc_out = [[nc.dram_tensor(f"agc_out{t}{q}", [4, 128, chunk_w(t, q)], bf16)
                for q in range(NQT)] for t in range(2)]
    rb_d = nc.dram_tensor("rb_bounce", [4, QT], f32)
    wrm_in = nc.dram_tensor("wrm_in", [1, SCC], bf16)
    wrm_out = nc.dram_tensor("wrm_out", [4, 1, SCC], bf16)
    if debug:
        dbgz_d = nc.dram_tensor("dbgz", [HPC, DH, S], f32, kind="ExternalOutput")
        dbgmr_d = nc.dram_tensor("dbgmr", [HPC, 2], f32, kind="ExternalOutput")
        dbgnr_d = nc.dram_tensor("dbgnr", [2, 128, 4, S], bf16, kind="ExternalOutput")

    with ExitStack() as ctx:
        tc = ctx.enter_context(tile.TileContext(nc))
        const = ctx.enter_context(tc.tile_pool(name="const", bufs=1))
        big = ctx.enter_context(tc.tile_pool(name="big", bufs=1))
        psc = ctx.enter_context(tc.tile_pool(name="psc", bufs=2, space="PSUM"))
        pav = ctx.enter_context(tc.tile_pool(name="pav", bufs=2, space="PSUM"))
        ppp = ctx.enter_context(tc.tile_pool(name="ppp", bufs=2, space="PSUM"))
        pexp = ctx.enter_context(tc.tile_pool(name="pexp", bufs=4))
        pd = ctx.enter_context(tc.tile_pool(name="pd", bufs=1))
        prb = ctx.enter_context(tc.tile_pool(name="prb", bufs=2))

        # ---- input DMAs: spread across engine queues for bandwidth ----
        wq_sb = const.tile([128, NDC, CW], bf16, tag="wq")
        wk_sb = const.tile([128, NDC, CW], bf16, tag="wk")
        wv_sb = const.tile([128, NDC, CW], bf16, tag="wv")
        wo_sb = const.tile([128, NDC, CW], bf16, tag="wo")
        nc.sync.dma_start(out=wq_sb, in_=wq_d[:, :].rearrange("p (c n) -> p c n", c=NDC))
        nc.gpsimd.dma_start(out=wk_sb, in_=wk_d[:, :].rearrange("p (c n) -> p c n", c=NDC))
        nc.scalar.dma_start(out=wv_sb, in_=wv_d[:, :].rearrange("p (c n) -> p c n", c=NDC))
        nc.scalar.dma_start(out=wo_sb, in_=wo_d[:, :].rearrange("p (c n) -> p c n", c=NDC))
        pxt = ctx.enter_context(tc.tile_pool(name="pxt", bufs=1))
        xt_sb = [pxt.tile([128, S], bf16, tag=f"xt{c}", name=f"xt{c}")
                 for c in range(NDC)]
        xt_engs = [nc.sync, nc.gpsimd, nc.scalar]
        for c in range(NDC):
            xt_engs[c % 3].dma_start(out=xt_sb[c], in_=xt_d[c * 128:(c + 1) * 128, :])

        # small constants on the gpsimd queue
        bqp_sb = const.tile([128, 2], f32, tag="bqp")
        bkp_sb = const.tile([128, 2], f32, tag="bkp")
        bvh_sb = const.tile([64, HPC], f32, tag="bvh")
        bvo_sb = const.tile([128, 2 * HPC], f32, tag="bvo")
        bor_sb = const.tile([1, CW], bf16, tag="bor")
        nc.gpsimd.dma_start(out=bqp_sb, in_=bqp_d[:, :])
        nc.gpsimd.dma_start(out=bkp_sb, in_=bkp_d[:, :])
        nc.gpsimd.dma_start(out=bvh_sb, in_=bvh_d[:, :])
        nc.gpsimd.dma_start(out=bvo_sb, in_=bvo_d[:, :])
        nc.gpsimd.dma_start(out=bor_sb, in_=bo_d[:].rearrange("(a n) -> a n", a=1))

        ones64 = const.tile([64, 1], f32, tag="ones64")
        nc.vector.memset(ones64, 1.0)
        ones1r = const.tile([1, 64], f32, tag="ones1r")
        nc.vector.memset(ones1r, 1.0)
        onesrow = const.tile([1, QT], bf16, tag="onesrow")
        nc.vector.memset(onesrow, 1.0)
        magic_sb = const.tile([1, 2], f32, tag="magic")
        nc.vector.memset(magic_sb, RSQRT_MAGIC)

        # pre-warm the collective fabric with a tiny gather
        wrm_sb = const.tile([1, SCC], bf16, tag="wrm")
        nc.vector.memset(wrm_sb, 0.0)
        nc.gpsimd.dma_start(out=wrm_in[0:1, :], in_=wrm_sb)
        if with_collective:
            nc.gpsimd.collective_compute(
                "AllGather", ALU.bypass, replica_groups=RG,
                ins=[wrm_in[:].opt()], outs=[wrm_out[:].opt()])

        # pre-warm the exp activation table while DMAs stream
        warm_sb = const.tile([1, 1], f32, tag="warm")
        nc.vector.memset(warm_sb, 0.0)
        warm2 = const.tile([1, 1], f32, tag="warm2")
        nc.scalar.activation(warm2, warm_sb, AF.Exp)

        qT_sb = big.tile([128, 2, S], bf16, tag="qT")   # pair t; head-parity o rows
        kT_sb = big.tile([128, 2, S], bf16, tag="kT")
        v_sb = [big.tile([128, NKT, DH + 1], bf16, tag=f"v{h}", name=f"v{h}")
                for h in range(HPC)]
        z_sb = [big.tile([DH, S], bf16, tag=f"z{h}", name=f"z{h}")
                for h in range(HPC)]
        nrmg = big.tile([128, NDC, S], bf16, tag="nrmg")   # chunk c=2g+t

        bnst = [pd.tile([64, NQT, 6], f32, tag=f"bn{h}", name=f"bnst{h}")
                for h in range(HPC)]
        stk_all = [pd.tile([64, 3], f32, tag=f"stk{h}", name=f"stk{h}")
                   for h in range(HPC)]
        mr_sb = {}
        state = {}

        def qk_proj_pair(t, st0, w_sb, bp_sb, dst):
            # two query-tile columns, c-outer so work starts on xt chunk 0
            ps = [ppp.tile([128, QT], f32, tag="pp",
                           name=f"qk{t}{st0}{j}{w_sb.tensor.name}")
                  for j in range(2)]
            for c in range(NDC):
                for j in range(2):
                    nc.tensor.matmul(ps[j], w_sb[:, c, t * 128:(t + 1) * 128],
                                     xt_sb[c][:, (st0 + j) * QT:(st0 + j + 1) * QT],
                                     start=(c == 0), stop=(c == NDC - 1))
            for j in range(2):
                nc.vector.tensor_scalar(
                    out=dst[:, t, (st0 + j) * QT:(st0 + j + 1) * QT],
                    in0=ps[j], scalar1=bp_sb[:, t:t + 1],
                    scalar2=None, op0=ALU.add)

        def qk_proj_st(t, st, w_sb, bp_sb, dst):
            ps = ppp.tile([128, QT], f32, tag="pp",
                          name=f"qk{t}{st}{w_sb.tensor.name}")
            for c in range(NDC):
                nc.tensor.matmul(ps, w_sb[:, c, t * 128:(t + 1) * 128],
                                 xt_sb[c][:, st * QT:(st + 1) * QT],
                                 start=(c == 0), stop=(c == NDC - 1))
            nc.vector.tensor_scalar(out=dst[:, t, st * QT:(st + 1) * QT],
                                    in0=ps, scalar1=bp_sb[:, t:t + 1],
                                    scalar2=None, op0=ALU.add)

        def v_proj_st(st, h01):
            # v for heads [2*h01, 2*h01+1] at key-tile st
            cs = slice(h01 * 2 * DH, (h01 + 1) * 2 * DH)
            ps = ppp.tile([128, QT], f32, tag="pp", name=f"v{h01}{st}")
            for c in range(NDC):
                nc.tensor.matmul(ps[:, 0:2 * DH],
                                 xt_sb[c][:, st * 128:(st + 1) * 128],
                                 wv_sb[:, c, cs],
                                 start=(c == 0), stop=(c == NDC - 1))
            for hh in range(2):
                h = 2 * h01 + hh
                nc.vector.tensor_copy(out=v_sb[h][:, st, 0:DH],
                                      in_=ps[:, hh * DH:(hh + 1) * DH])

        def pair_stats(t):
            # bn_aggr + cross-partition combine + Newton rsqrt -> (M', r)
            h0, h1 = 2 * t, 2 * t + 1
            scg = pd.tile([1, 2, 3], f32, tag=f"scg{t}", name=f"scg{t}")
            for i, h in enumerate((h0, h1)):
                mvh = pd.tile([64, 2], f32, tag="mv", bufs=2, name=f"mv{h}")
                nc.vector.bn_aggr(out=mvh, in_=bnst[h])
                stk = stk_all[h]
                nc.vector.tensor_add(stk[:, 0:1], mvh[:, 0:1], bvh_sb[:, h:h + 1])
                nc.vector.tensor_copy(stk[:, 1:2], mvh[:, 1:2])
                nc.vector.tensor_mul(stk[:, 2:3], stk[:, 0:1], stk[:, 0:1])
                stp = ppp.tile([1, 3], f32, tag="pp", name=f"stp{h}")
                nc.tensor.matmul(stp, ones64, stk, start=True, stop=True)
                nc.vector.tensor_scalar(out=scg[:, i, :], in0=stp,
                                        scalar1=1.0 / 64.0, scalar2=None,
                                        op0=ALU.mult)
            # var_tot = E[var] + E[(m+bv)^2] - M'^2 ; r = rsqrt(var_tot + eps)
            m2 = pd.tile([1, 2], f32, tag=f"m2{t}", name=f"m2{t}")
            nc.vector.tensor_mul(m2, scg[:, :, 0], scg[:, :, 0])
            vr = pd.tile([1, 2], f32, tag=f"vr{t}", name=f"vr{t}")
            nc.vector.tensor_add(vr, scg[:, :, 1], scg[:, :, 2])
            nc.vector.tensor_tensor(out=vr, in0=vr, in1=m2, op=ALU.subtract)
            nc.vector.tensor_scalar(out=vr, in0=vr, scalar1=EPS, scalar2=None,
                                    op0=ALU.add)
            yr = pd.tile([1, 2], f32, tag=f"yr{t}", name=f"yr{t}")
            ish = pd.tile([1, 2], i32, tag=f"ish{t}", name=f"ish{t}")
            nc.vector.tensor_scalar(out=ish, in0=vr[:, :].bitcast(i32),
                                    scalar1=1, scalar2=None,
                                    op0=ALU.logical_shift_right)
            nc.vector.tensor_tensor(
                out=yr[:, :].bitcast(i32), in0=magic_sb[:, :].bitcast(i32),
                in1=ish, op=ALU.subtract)
            tt = pd.tile([1, 2], f32, tag=f"tt{t}", name=f"tt{t}")
            for _ in range(2):
                nc.vector.tensor_mul(tt, yr, yr)
                nc.vector.tensor_mul(tt, tt, vr)
                nc.vector.tensor_scalar(out=tt, in0=tt, scalar1=-0.5,
                                        scalar2=1.5, op0=ALU.mult, op1=ALU.add)
                nc.vector.tensor_mul(yr, yr, tt)
            for i, h in enumerate((h0, h1)):
                mr = pd.tile([1, 2], f32, tag="mr", bufs=4, name=f"mr{h}")
                nc.vector.tensor_copy(mr[:, 0:1], scg[:, i, 0:1])
                nc.vector.tensor_copy(mr[:, 1:2], yr[:, i:i + 1])
                mr_sb[h] = mr
            # payload [M_h0, r_h0, M_h1, r_h1]
            msc = pd.tile([1, 4], f32, tag=f"msc{t}", name=f"msc{t}")
            for i, h in enumerate((h0, h1)):
                nc.vector.tensor_copy(msc[:, 2 * i:2 * i + 1], mr_sb[h][:, 0:1])
                nc.vector.tensor_copy(msc[:, 2 * i + 1:2 * i + 2],
                                      mr_sb[h][:, 1:2])
            state[f"msc{t}"] = msc

        def gather(in_t, out_t):
            if with_collective:
                nc.gpsimd.collective_compute(
                    "AllGather", ALU.bypass, replica_groups=RG,
                    ins=[in_t[:].opt()], outs=[out_t[:].opt()])
            else:
                for g in range(4):
                    nc.sync.dma_start(out=out_t[g], in_=in_t[:, :])

        def stage_chunk(t, q):
            # gathered chunk -> nrmg[:, 2g+t, q-slice]; then payload extract
            nc.gpsimd.dma_start(
                out=nrmg[:, t::2, q * QT:(q + 1) * QT],
                in_=agc_out[t][q][:, :, 0:QT].rearrange("g p q -> p g q"))
            if q == 3:
                sc = pd.tile([1, 4, SCC], bf16, tag=f"sc16{t}", name=f"sc16{t}")
                nc.gpsimd.dma_start(
                    out=sc,
                    in_=agc_out[t][3][:, 0:1, QT:QT + SCC].rearrange("g p c -> p g c"))
                state[f"sc16{t}"] = sc

        qk1_work = []
        for st in range(NQT):
            qk1_work.append(("q", st))
            qk1_work.append(("k", st))

        SEQA = [(t, qt, kt) for t in (0, 1) for qt in range(NQT)
                for kt in range(NKT)]
        e_tiles = {}
        av_cur = {}

        def emit_scores_exp(j):
            t, qt, kt = SEQA[j]
            sps = psc.tile([128, 2 * QT], f32, tag="s", name=f"s{t}{qt}{kt}")
            for o in range(2):
                nc.tensor.matmul(
                    sps[:, o * QT:(o + 1) * QT],
                    kT_sb[64 * o:64 * (o + 1), t, kt * 128:(kt + 1) * 128],
                    qT_sb[64 * o:64 * (o + 1), t, qt * QT:(qt + 1) * QT],
                    start=True, stop=True)
            e_sb = pexp.tile([128, 2 * QT], bf16, tag="e", name=f"e{t}{qt}{kt}")
            nc.scalar.activation(e_sb, sps, AF.Exp)
            e_tiles[j] = e_sb

        def attn_iter(j):
            t, qt, kt = SEQA[j]
            h0, h1 = 2 * t, 2 * t + 1
            if j + 1 < len(SEQA):
                emit_scores_exp(j + 1)
            if kt == 0:
                av_cur[0] = pav.tile([DH + 1, QT], f32, tag="av", name=f"av{t}{qt}a")
                av_cur[1] = pav.tile([DH + 1, QT], f32, tag="av", name=f"av{t}{qt}b")
            av0, av1 = av_cur[0], av_cur[1]
            e_sb = e_tiles.pop(j)
            nc.tensor.matmul(av0, v_sb[h0][:, kt, :], e_sb[:, 0:QT],
                             start=(kt == 0), stop=(kt == NKT - 1))
            nc.tensor.matmul(av1, v_sb[h1][:, kt, :], e_sb[:, QT:2 * QT],
                             start=(kt == 0), stop=(kt == NKT - 1))
            if t == 0:
                # sprinkle pair-1 projections into the exp-bound loop
                if kt % 4 == 3:
                    v_proj_st(qt * 4 + (kt - 3) // 4, 1)
                elif kt % 8 == 5 and qk1_work:
                    kind, pst = qk1_work.pop(0)
                    if kind == "q":
                        qk_proj_st(1, pst, wq_sb, bqp_sb, qT_sb)
                    else:
                        qk_proj_st(1, pst, wk_sb, bkp_sb, kT_sb)
                if (qt, kt) == (3, 15):
                    for h in range(2, 4):
                        nc.vector.memset(v_sb[h][:, :, DH:DH + 1], 1.0)
            if kt < NKT - 1:
                return
            # ---- end of qt: normalize, stats, chunk gather ----
            for i, (h, av) in enumerate(((h0, av0), (h1, av1))):
                row = 2 * (qt % 2) + i
                zt = prb.tile([DH + 1, QT], f32, tag="zt", name=f"zt{t}{qt}{i}")
                nc.vector.tensor_copy(out=zt, in_=av)
                nc.sync.dma_start(out=rb_d[row:row + 1, :],
                                  in_=zt[DH:DH + 1, :])
                rb = prb.tile([64, QT], f32, tag="rb", name=f"rb{t}{qt}{i}")
                nc.sync.dma_start(out=rb,
                                  in_=rb_d[row:row + 1, :].to_broadcast([64, QT]))
                nc.vector.reciprocal_approx_fast(rb, rb)
                zsl = z_sb[h][:, qt * QT:(qt + 1) * QT]
                nc.vector.tensor_mul(zsl, zt[0:DH, :], rb)
                nc.vector.bn_stats(out=bnst[h][:, qt, :], in_=zsl)
            nc.gpsimd.dma_start(out=agc_in[t][qt][0:64, 0:QT],
                              in_=z_sb[h0][:, qt * QT:(qt + 1) * QT])
            nc.gpsimd.dma_start(out=agc_in[t][qt][64:128, 0:QT],
                              in_=z_sb[h1][:, qt * QT:(qt + 1) * QT])
            if qt == 3:
                pair_stats(t)
                nc.gpsimd.dma_start(
                    out=agc_in[t][3][0:1, QT:QT + SCC],
                    in_=state[f"msc{t}"][0:1, :].bitcast(bf16))
            gather(agc_in[t][qt], agc_out[t][qt])
            stage_chunk(t, qt)

        # ---- lead: q/k st0-1 c-interleaved in psc halves, early pipeline
        qk01 = psc.tile([128, 2 * QT], f32, tag="s", name="qk01q")
        kk01 = psc.tile([128, 2 * QT], f32, tag="s", name="qk01k")
        for c in range(NDC):
            for j in range(2):
                nc.tensor.matmul(qk01[:, j * QT:(j + 1) * QT],
                                 wq_sb[:, c, 0:128],
                                 xt_sb[c][:, j * QT:(j + 1) * QT],
                                 start=(c == 0), stop=(c == NDC - 1))
            for j in range(2):
                nc.tensor.matmul(kk01[:, j * QT:(j + 1) * QT],
                                 wk_sb[:, c, 0:128],
                                 xt_sb[c][:, j * QT:(j + 1) * QT],
                                 start=(c == 0), stop=(c == NDC - 1))
        for j in range(2):
            nc.vector.tensor_scalar(out=qT_sb[:, 0, j * QT:(j + 1) * QT],
                                    in0=qk01[:, j * QT:(j + 1) * QT],
                                    scalar1=bqp_sb[:, 0:1],
                                    scalar2=None, op0=ALU.add)
            nc.vector.tensor_scalar(out=kT_sb[:, 0, j * QT:(j + 1) * QT],
                                    in0=kk01[:, j * QT:(j + 1) * QT],
                                    scalar1=bkp_sb[:, 0:1],
                                    scalar2=None, op0=ALU.add)
        emit_scores_exp(0)
        qk_proj_pair(0, 2, wq_sb, bqp_sb, qT_sb)
        qk_proj_pair(0, 2, wk_sb, bkp_sb, kT_sb)
        for st in range(NKT):
            v_proj_st(st, 0)
        for h in range(2):
            nc.vector.memset(v_sb[h][:, :, DH:DH + 1], 1.0)

        # receiver maps for pair-0 chunks (payload lands mid-pair-1)
        def build_maps(t, sc, pool, ptag):
            scf = sc[:, :, :].bitcast(f32)     # [1, 4, 4]: M0 r0 M1 r1
            mm = pool.tile([128, 4], f32, tag=ptag, name=f"mapm{t}")
            rr = pool.tile([128, 4], f32, tag=ptag, name=f"mapr{t}")
            for o in range(2):
                nc.tensor.matmul(mm[64 * o:64 * (o + 1), :], ones1r,
                                 scf[:, :, 2 * o], start=True, stop=True)
                nc.tensor.matmul(rr[64 * o:64 * (o + 1), :], ones1r,
                                 scf[:, :, 2 * o + 1], start=True, stop=True)
            rmap = pg.tile([128, 4], f32, tag=f"rmap{t}")
            nc.vector.tensor_copy(rmap, rr)
            mvec = pg.tile([128, 4], bf16, tag=f"mvec{t}")
            mtmp = pg.tile([128, 4], f32, tag=f"mtmp{t}")
            nc.vector.tensor_tensor(out=mtmp, in0=bvo_sb[:, t::2], in1=mm,
                                    op=ALU.subtract)
            nc.vector.tensor_mul(mvec, mtmp, rmap)
            wos = pg.tile([128, 4, CW], bf16, tag=f"wos{t}")
            for g in range(4):
                nc.vector.tensor_scalar(out=wos[:, g, :],
                                        in0=wo_sb[:, 2 * g + t, :],
                                        scalar1=rmap[:, g:g + 1], scalar2=None,
                                        op0=ALU.mult)
            return mvec, wos

        pg = ctx.enter_context(tc.tile_pool(name="pg", bufs=1))
        for j in range(len(SEQA)):
            attn_iter(j)
        mvec0, wos0 = build_maps(0, state["sc160"], ppp, "pp")

        if debug:
            for h in range(HPC):
                nc.gpsimd.dma_start(out=dbgz_d[h], in_=z_sb[h])
                nc.sync.dma_start(out=dbgmr_d[h:h + 1, :], in_=mr_sb[h])
            nc.sync.dma_start(out=dbgnr_d[0], in_=nrmg[:, 0::2, :])
            nc.sync.dma_start(out=dbgnr_d[1], in_=nrmg[:, 1::2, :])

        # ---- tail: pair-1 maps, bias row, out-projection ----
        with tc.tile_pool(name="pystage", bufs=1) as pystage:
            ystage = [pystage.tile([128, S], bf16, tag=f"ys{nt}", name=f"ys{nt}")
                      for nt in range(2)]
            # pair-0 accumulation runs while the last gather is in flight
            yp0 = [psc.tile([128, 2 * QT], f32, tag="s", name=f"yp0_{j}")
                   for j in range(2)]
            for g in range(4):
                for st in range(NQT):
                    nc.tensor.matmul(
                        yp0[st // 2][:, (st % 2) * QT:(st % 2 + 1) * QT],
                        wos0[:, g, 0:128],
                        nrmg[:, 2 * g, st * QT:(st + 1) * QT],
                        start=(g == 0), stop=False)
            yp1a = [ppp.tile([128, QT], f32, tag="pp", name=f"yp1a_{st}")
                    for st in range(2)]
            for g in range(4):
                for st in range(2):
                    nc.tensor.matmul(
                        yp1a[st],
                        wos0[:, g, 128:256],
                        nrmg[:, 2 * g, st * QT:(st + 1) * QT],
                        start=(g == 0), stop=False)

            # keep the PE array (and HAM clock) busy while the final
            # gather + scalar payload are still in flight
            for wi in range(100):
                dmy = pav.tile([DH + 1, QT], f32, tag="av", name=f"dmy{wi}")
                nc.tensor.matmul(dmy, v_sb[0][:, 0, :], nrmg[:, 0, 0:QT],
                                 start=True, stop=True)
            mvec1, wos1 = build_maps(1, state["sc161"], pav, "av")

            # bias row: bo + sum_d (bv-M)*r*wo over all chunks
            cstp = pav.tile([1, CW], f32, tag="av", name="cstp")
            for t, mv in ((0, mvec0), (1, mvec1)):
                for g in range(4):
                    nc.tensor.matmul(cstp, mv[:, g:g + 1], wo_sb[:, 2 * g + t, :],
                                     start=(t == 0 and g == 0),
                                     stop=(t == 1 and g == 3))
            brow = pg.tile([1, CW], bf16, tag="brow")
            nc.vector.tensor_tensor(out=brow, in0=bor_sb, in1=cstp, op=ALU.add)

            def st_epilogue(nt, st, src_ap):
                nc.tensor.matmul(src_ap, brow[:, nt * 128:(nt + 1) * 128],
                                 onesrow, start=False, stop=True)
                dst = ystage[nt][:, st * QT:(st + 1) * QT]
                if st % 2 == 0:
                    nc.scalar.activation(dst, src_ap, AF.Copy)
                else:
                    nc.vector.tensor_copy(out=dst, in_=src_ap)
                nc.sync.dma_start(out=y_d[nt, :, st * QT:(st + 1) * QT],
                                  in_=ystage[nt][:, st * QT:(st + 1) * QT])

            # nt0: pair-1 chunks st-outer with per-st drain
            for st in range(NQT):
                src_ap = yp0[st // 2][:, (st % 2) * QT:(st % 2 + 1) * QT]
                for g in range(4):
                    nc.tensor.matmul(src_ap, wos1[:, g, 0:128],
                                     nrmg[:, 2 * g + 1, st * QT:(st + 1) * QT],
                                     start=False, stop=False)
                st_epilogue(0, st, src_ap)
            # nt1 st0-1
            for st in range(2):
                for g in range(4):
                    nc.tensor.matmul(yp1a[st], wos1[:, g, 128:256],
                                     nrmg[:, 2 * g + 1, st * QT:(st + 1) * QT],
                                     start=False, stop=False)
                st_epilogue(1, st, yp1a[st])
            # nt1 st2-3 (sc-tag psum frees after nt0 drains)
            yp1b = psc.tile([128, 2 * QT], f32, tag="s", name="yp1b")
            for st in range(2, NQT):
                sl = yp1b[:, (st - 2) * QT:(st - 1) * QT]
                for g in range(4):
                    nc.tensor.matmul(sl, wos0[:, g, 128:256],
                                     nrmg[:, 2 * g, st * QT:(st + 1) * QT],
                                     start=(g == 0), stop=False)
                for g in range(4):
                    nc.tensor.matmul(sl, wos1[:, g, 128:256],
                                     nrmg[:, 2 * g + 1, st * QT:(st + 1) * QT],
                                     start=False, stop=False)
                st_epilogue(1, st, sl)

    nc.compile()
    return nc


def _get_nc():
    if "nc" not in _cache:
        _cache["nc"] = _build()
    return _cache["nc"]


def _host_prep(x, Wq, bq, Wk, bk, Wv, bv, Wo, bo, lq1, lk1, lq2, lk2, gn_w, gn_b):
    x = np.asarray(x, np.float32)
    lam = (np.exp((np.asarray(lq1) * np.asarray(lk1)).sum(-1))
           - np.exp((np.asarray(lq2) * np.asarray(lk2)).sum(-1)) + LAMBDA_INIT)
    qscale = (DH ** -0.5) * lam
    Wq_eff = (np.asarray(Wq).reshape(D, H, DH) * qscale[None, :, None]).reshape(D, D)
    bq_eff = (np.asarray(bq).reshape(H, DH) * qscale[:, None]).reshape(D)
    gw = np.asarray(gn_w).reshape(D)
    gb = np.asarray(gn_b).reshape(D)
    Wo_eff = np.asarray(Wo) * gw[:, None]
    bo_eff = np.asarray(bo) + gb @ np.asarray(Wo)
    bk_full = np.asarray(bk)
    bv_full = np.asarray(bv, np.float32)

    # Gathered-row order (chunk (g,t), partition (o,dh) -> head 4g+2t+o) is
    # exactly the original row-major head order, so Wo_eff rows need no
    # permutation.
    xT = np.ascontiguousarray(x.transpose(0, 2, 1))  # [B, D, S]
    bf = ml_dtypes.bfloat16

    def pair_partition_layout(vec256):
        # [256] (head-major: (2t+o)*64+dh) -> [128, 2] with row o*64+dh, col t
        return np.ascontiguousarray(
            vec256.reshape(2, 2, DH).transpose(1, 2, 0).reshape(128, 2)
        ).astype(np.float32)

    # receiver bv map: bvo[o*64+dh, 2g+t] = bv[(4g+2t+o)*64+dh]
    bvo = np.ascontiguousarray(
        bv_full.reshape(4, 2, 2, DH).transpose(2, 3, 0, 1).reshape(128, 8)
    ).astype(np.float32)

    in_maps = []
    for c in range(N_CORES):
        b, hg = c // 4, c % 4
        cs = slice(CW * hg, CW * (hg + 1))
        def wlay(w):
            # [D, CW] -> [128, NDC*CW] with row=partition, chunks contiguous
            return np.ascontiguousarray(
                w.reshape(NDC, 128, CW).transpose(1, 0, 2).reshape(128, NDC * CW)
            ).astype(bf)

        in_maps.append({
            "xt": np.ascontiguousarray(xT[b]).astype(bf),
            "wq": wlay(Wq_eff[:, cs]),
            "wk": wlay(np.asarray(Wk)[:, cs]),
            "wv": wlay(np.asarray(Wv)[:, cs]),
            "wo": wlay(Wo_eff[:, cs]),
            "bqp": pair_partition_layout(bq_eff[cs]),
            "bkp": pair_partition_layout(bk_full[cs]),
            "bvh": np.ascontiguousarray(
                bv_full[cs].reshape(HPC, DH).T).astype(np.float32),
            "bvo": bvo,
            "bo": np.ascontiguousarray(bo_eff[cs]).astype(bf),
        })
    return in_maps


def _host_gather(outs):
    # core c=4b+hg produced output columns [256*hg, 256*(hg+1)) as [2,128,S]
    yT = np.empty((B, D, S), np.float32)
    for b in range(B):
        for hg in range(4):
            q = np.asarray(outs[4 * b + hg]["y"], np.float32).reshape(CW, S)
            yT[b, CW * hg:CW * (hg + 1), :] = q
    return np.ascontiguousarray(yT.transpose(0, 2, 1))


def kernel(x, Wq, bq, Wk, bk, Wv, bv, Wo, bo, lq1, lk1, lq2, lk2, gn_w, gn_b):
    from concourse.bass_utils import run_bass_kernel_spmd

    in_maps = _host_prep(x, Wq, bq, Wk, bk, Wv, bv, Wo, bo,
                         lq1, lk1, lq2, lk2, gn_w, gn_b)
    nc = _get_nc()
    res = run_bass_kernel_spmd(nc, in_maps, core_ids=list(range(N_CORES)))
    return _host_gather(res.results)
